# revision 30
# baseline (speedup 1.0000x reference)
"""Multi-head attention Trainium2 Bass kernel, sharded over 8 NeuronCores.

Problem: B=4, S=2048, D=1024, H=16 heads (DK=64), fp32, random 0/1 mask.

Sharding (data-parallel batch x tensor-parallel heads):
  core c handles batch b = c // 2, head-group hg = c % 2 (8 heads = 512 dims).
  Each core computes Q/K/V projections for its head-group, masked softmax
  attention for its 8 heads, and a partial output projection over its 512
  contraction dims. The host sums the two partials per batch (the "all-reduce"
  is a host-side pairwise add since we gather outputs anyway).

On-device layout (per core):
  All matmuls run as float32r (full PE rate at N=512, ~1e-3 relative precision)
  except attention-weights @ V which runs in bf16 (E and V tiles), because the
  mask multiply on the vector engine needs bf16 for its 2x mode.

  K^T is kept resident [512, 2048] (head dim on partitions); Q^T is computed
  per 512-wide q-chunk inside the attention loop (hides the Q projection under
  the exp-bound phase). Scores come out transposed, S^T = [k, q], with the two
  heads of a pair row-packed into disjoint halves of the PE array. The softmax
  sum over k rides the P@V matmul as a ones column appended to V (row 64 of
  the C' accumulator = sum_k E_masked). exp() runs on the scalar engine out of
  PSUM (2 banks per ACTIVATE); the mask multiply runs on the vector engine in
  bf16 2x mode; 1/Z = scalar-engine partition-shifting copy (PSUM row 64 ->
  SBUF row 0) + reciprocal_approx_fast + gpsimd partition_broadcast. The
  output projection of chunk qc is emitted inside chunk qc+1's loop so its
  matmuls never stall the in-order PE queue.

  Output is produced transposed ([1024, 2048] per core); host re-transposes
  and sums the two head-group partials per batch.
"""
import numpy as np

import concourse.bass as bass
import concourse.mybir as mybir
import concourse.tile as tile
from concourse import bacc

B, S, D, H = 4, 2048, 1024, 16
DK = D // H          # 64
NCORES = 8
HG = 2               # head groups (tensor-parallel factor per batch)
HPG = H // HG        # 8 heads per core
DH = D // HG         # 512 head dims per core
QCN = 4              # q chunks
QCS = S // QCN       # 512
KT = S // 128        # 16 k tiles
DT = D // 128        # 8 contraction tiles for projections
F32 = mybir.dt.float32
F32R = mybir.dt.float32r
BF16 = mybir.dt.bfloat16

# k-tile grouping for the exp pass (PSUM banks per S^T buffer)
EXP_GROUPS = [3, 3, 3, 3, 2, 2]
assert sum(EXP_GROUPS) == KT


def r(ap):
    """Matmul operands are stored as float32r already; identity."""
    return ap


def build_nc():
    nc = bacc.Bacc(None)
    xqT = nc.declare_dram_parameter("xqT", [D, S], F32R, isOutput=False)
    xkT = nc.declare_dram_parameter("xkT", [D, S], F32R, isOutput=False)
    xvT = nc.declare_dram_parameter("xvT", [D, S], F32R, isOutput=False)
    maskT = nc.declare_dram_parameter("maskT", [S, S], BF16, isOutput=False)
    wqT = nc.declare_dram_parameter("wqT", [D, DH], F32R, isOutput=False)
    wkT = nc.declare_dram_parameter("wkT", [D, DH], F32R, isOutput=False)
    wvT = nc.declare_dram_parameter("wvT", [D, DH], F32R, isOutput=False)
    woT = nc.declare_dram_parameter("woT", [DH, D], F32R, isOutput=False)
    bq2 = nc.declare_dram_parameter("bq2", [128, DH // 128], F32, isOutput=False)
    bk2 = nc.declare_dram_parameter("bk2", [128, DH // 128], F32, isOutput=False)
    vr2 = nc.declare_dram_parameter("vr2", [128, D // 128], F32, isOutput=False)
    outT = nc.declare_dram_parameter("outT", [D, S], F32, isOutput=True)

    with tile.TileContext(nc) as tc:
        with tc.tile_pool(name="persist", bufs=1) as pp:
            # ---- persistent tiles ----
            wo_full = pp.tile([128, DH // 128, D], F32R, tag="wo", name="wo_full")
            wo_sb = [wo_full[:, i, :] for i in range(DH // 128)]
            bias_sb = pp.tile([128, 2 * (DH // 128) + D // 128], F32, tag="bias",
                              name="bias_sb")
            bq_sb = bias_sb[:, 0:DH // 128]
            bk_sb = bias_sb[:, DH // 128:2 * (DH // 128)]
            vr_sb = bias_sb[:, 2 * (DH // 128):]
            wq_full = pp.tile([128, DT, DH], F32R, tag="wq", name="wq_full")
            wq_sb = [wq_full[:, i, :] for i in range(DT)]
            kt_sb = [pp.tile([128, S], F32R, tag=f"kt{i}", name=f"kt{i}")
                     for i in range(DH // 128)]
            v_full = pp.tile([128, KT, HPG * 65], BF16, tag="v", name="v_full")
            v_sb = [v_full[:, i, :] for i in range(KT)]

            # ---- phase A: projections ----
            with (
                tc.tile_pool(name="w_a", bufs=1) as wpool,
                tc.tile_pool(name="x_a", bufs=2) as xpool,
                tc.tile_pool(name="ps_a", bufs=6, space="PSUM") as pspool,
            ):
                wk_full = wpool.tile([128, DT, DH], F32R, tag="wk", name="wk_full")
                wv_full = wpool.tile([128, DT, DH], F32R, tag="wv", name="wv_full")
                wk_sb = [wk_full[:, i, :] for i in range(DT)]
                wv_sb = [wv_full[:, i, :] for i in range(DT)]
                # wk first on the sync queue (K projection runs first)
                for i in range(DT):
                    nc.sync.dma_start(wk_sb[i][:], wkT[i * 128:(i + 1) * 128, :])
                nc.gpsimd.dma_start(bq_sb[:, :], bq2[:])
                nc.gpsimd.dma_start(bk_sb[:, :], bk2[:])
                nc.gpsimd.dma_start(vr_sb[:, :], vr2[:])
                for i in range(KT):
                    ones_cols = v_sb[i].rearrange("p (h c) -> p h c", h=HPG)[:, :, 64:65]
                    nc.gpsimd.memset(ones_cols, 1.0)

                def qk_proj(xT_dram, w_sb, dst_tiles, scale, bias_sb, wtag,
                            first_on_pool=False):
                    for qc in range(QCN):
                        x_t = xpool.tile([128, DT, QCS], F32R, tag="x", name=f"x_{wtag}",
                                         bufs=2)
                        xs = xT_dram[:, qc * QCS:(qc + 1) * QCS].rearrange(
                            "(t p) s -> p t s", p=128)
                        for i in range(DT):
                            eng = nc.gpsimd if (first_on_pool or i % 2) else nc.sync
                            eng.dma_start(x_t[:, i:i + 1, :], xs[:, i:i + 1, :])
                        first_on_pool = False
                        ps_l = [pspool.tile([128, QCS], F32, tag="ps",
                                            name=f"ps_{wtag}{dt}")
                                for dt in range(DH // 128)]
                        for i in range(DT):
                            for dt in range(DH // 128):
                                nc.tensor.matmul(
                                    ps_l[dt][:], w_sb[i][:, dt * 128:(dt + 1) * 128],
                                    x_t[:, i, :], start=(i == 0), stop=(i == DT - 1))
                        for dt in range(DH // 128):
                            nc.vector.tensor_scalar(
                                dst_tiles[dt][:, qc * QCS:(qc + 1) * QCS], ps_l[dt][:],
                                scale, bias_sb[:, dt:dt + 1],
                                mybir.AluOpType.mult, mybir.AluOpType.add)

                qk_proj(xkT, wk_sb, kt_sb, 1.0, bk_sb, "k", first_on_pool=True)
                for i in range(DT):
                    nc.sync.dma_start(wv_sb[i][:], wvT[i * 128:(i + 1) * 128, :])
                for i in range(DT):
                    nc.sync.dma_start(wq_sb[i][:], wqT[i * 128:(i + 1) * 128, :])

                def qproj_b(qc, xp, xtag, psp, pstag):
                    qt_t = pp.tile([128, DH // 128, QCS], F32R, tag="qt",
                                   name="qt_t", bufs=2)
                    xq2 = [xp.tile([128, DT // 2, QCS], F32R, tag=xtag, name="xq2",
                                   bufs=2) for _ in range(2)]
                    for c in range(2):
                        xs = xqT[c * (D // 2):(c + 1) * (D // 2),
                                 qc * QCS:(qc + 1) * QCS].rearrange(
                            "(t p) s -> p t s", p=128)
                        nc.sync.dma_start(xq2[c][:], xs)
                    for dt in range(DH // 128):
                        ps = psp.tile([128, QCS], F32, tag=pstag, name="ps_q")
                        for i in range(DT):
                            nc.tensor.matmul(
                                ps[:], wq_sb[i][:, dt * 128:(dt + 1) * 128],
                                xq2[i // (DT // 2)][:, i % (DT // 2), :],
                                start=(i == 0), stop=(i == DT - 1))
                        nc.vector.tensor_scalar(
                            qt_t[:, dt, :], ps[:], 0.125, bq_sb[:, dt:dt + 1],
                            mybir.AluOpType.mult, mybir.AluOpType.add)
                    qt_tiles[qc] = qt_t

                qt_tiles = {}

                for st4 in range(KT // 4):
                    xv4 = xpool.tile([128, DT, QCS], F32R, tag="x", name="xv4", bufs=2)
                    nc.gpsimd.dma_start(
                        xv4[:],
                        xvT[:, st4 * QCS:(st4 + 1) * QCS].rearrange(
                            "(t p) s -> p t s", p=128))
                    for sub in range(4):
                        st = st4 * 4 + sub
                        ps = pspool.tile([128, DH], F32, tag="ps", name="ps_v")
                        for i in range(DT):
                            nc.tensor.matmul(
                                ps[:], xv4[:, i, sub * 128:(sub + 1) * 128],
                                wv_sb[i][:], start=(i == 0), stop=(i == DT - 1))
                        vdst = v_sb[st].rearrange("p (h c) -> p h c", h=HPG)[:, :, 0:64]
                        nc.vector.tensor_copy(
                            vdst, ps[:].rearrange("p (h c) -> p h c", h=HPG))
                qproj_b(0, xpool, "x", pspool, "ps")

            # wo loads can land any time before the first output projection
            for i in range(DH // 128):
                nc.gpsimd.dma_start(wo_sb[i][:], woT[i * 128:(i + 1) * 128, :])

            # ---- phase B: attention + pipelined output projection ----
            with (
                tc.tile_pool(name="work", bufs=2) as wp,
                tc.tile_pool(name="psS", bufs=2, space="PSUM") as psS,
                tc.tile_pool(name="psC", bufs=4, space="PSUM") as psC,
            ):
                prev = None  # (cpair tiles, qc) pending output projection

                def emit_outproj(cpair, qc, ots, tail=False):
                    for ot in ots:
                        po = psC.tile([128, QCS], F32, tag="cps", name="po")
                        for j in range(HPG // 2):
                            nc.tensor.matmul(
                                po[:], wo_sb[j][:, ot * 128:(ot + 1) * 128],
                                cpair[j][:],
                                start=(j == 0), stop=(j == HPG // 2 - 1))
                        o_sb = wp.tile([128, QCS], F32, tag="o", name="o_sb", bufs=2)
                        if tail and ot % 2 == 0:
                            nc.scalar.activation(
                                o_sb[:], po[:], mybir.ActivationFunctionType.Identity,
                                bias=vr_sb[:, ot:ot + 1])
                        else:
                            nc.vector.tensor_scalar(
                                o_sb[:], po[:], 1.0, vr_sb[:, ot:ot + 1],
                                mybir.AluOpType.mult, mybir.AluOpType.add)
                        eng = nc.gpsimd if ot % 2 else nc.sync
                        eng.dma_start(
                            outT[ot * 128:(ot + 1) * 128, qc * QCS:(qc + 1) * QCS],
                            o_sb[:])

                def load_mask(qc):
                    mask_sb = wp.tile([128, KT, QCS], BF16, tag="mask", name="mask_sb",
                                      bufs=2)
                    ms = maskT[:, qc * QCS:(qc + 1) * QCS].rearrange(
                        "(t p) s -> p t s", p=128)
                    hm = KT // 2
                    nc.sync.dma_start(mask_sb[:, 0:hm, :], ms[:, 0:hm, :])
                    nc.gpsimd.dma_start(mask_sb[:, hm:KT, :], ms[:, hm:KT, :])
                    return mask_sb

                mask_next = load_mask(0)
                for qc in range(QCN):
                    mask_sb = mask_next
                    qt_cur = qt_tiles.pop(qc)
                    cpair_t = wp.tile([128, HPG // 2, QCS], F32R, tag="cp",
                                      name="cpair_t", bufs=2)
                    cpair = [cpair_t[:, j, :] for j in range(HPG // 2)]
                    for j in range(HPG // 2):
                        dtile = j
                        cps = [psC.tile([128, QCS], F32, tag="cps", name=f"cps{hh}",
                                        bufs=4) for hh in range(2)]
                        for kt in range(KT):
                            sps = psS.tile([128, 2, QCS], F32, tag="sps", name="sps",
                                           bufs=2)
                            for hh in range(2):
                                prow = hh * 64
                                nc.tensor.matmul(
                                    sps[:, hh, :],
                                    kt_sb[dtile][prow:prow + 64, kt * 128:(kt + 1) * 128],
                                    qt_cur[prow:prow + 64, dtile, :],
                                    start=True, stop=True)
                            e_sb = wp.tile([128, 2, QCS], BF16, tag="e", name="e_sb",
                                           bufs=3)
                            nc.scalar.activation(
                                e_sb[:], sps[:],
                                mybir.ActivationFunctionType.Exp)
                            meng = nc.gpsimd if kt % 4 == 3 else nc.vector
                            for hh in range(2):
                                meng.tensor_mul(
                                    e_sb[:, hh, :], e_sb[:, hh, :], mask_sb[:, kt, :])
                            for hh in range(2):
                                h = 2 * j + hh
                                nc.tensor.matmul(
                                    cps[hh][0:65, :],
                                    v_sb[kt][:, h * 65:(h + 1) * 65],
                                    e_sb[:, hh, :],
                                    start=(kt == 0), stop=(kt == KT - 1))
                        for hh in (1, 0):
                            # normalize: C[d, q] / Z[q]; Z = PSUM row 64.
                            # The scalar engine does a partition-SHIFTING copy
                            # (PSUM row 64 -> SBUF row 0), reciprocal runs at
                            # partition 0 (custom-DVE PSUM reads at partition
                            # 64 are broken on HW), then broadcast + multiply.
                            ns = wp.tile([64, 2, QCS], F32, tag="ns", name="ns", bufs=2)
                            nc.scalar.copy(ns[0:1, 0, :], cps[hh][64:65, :])
                            nc.vector.reciprocal_approx_fast(
                                out=ns[0:1, 1, :], in_=ns[0:1, 0, :])
                            rb = ns[0:64, 0, :]
                            nc.gpsimd.partition_broadcast(rb, ns[0:1, 1, :],
                                                          channels=64)
                            if hh == 0:
                                nc.vector.tensor_mul(cpair[j][0:64, :],
                                                     cps[hh][0:64, :], rb)
                            else:
                                cstage = wp.tile([64, QCS], F32R, tag="cstage",
                                                 name="cstage", bufs=2)
                                nc.vector.tensor_mul(cstage[:], cps[hh][0:64, :], rb)
                                nc.sync.dma_start(cpair[j][64:128, :], cstage[:])
                        if prev is not None:
                            emit_outproj(prev[0], prev[1], range(2 * j, 2 * j + 2))
                        if j == 0 and qc + 1 < QCN:
                            mask_next = load_mask(qc + 1)
                        if j == 1 and qc + 1 < QCN:
                            qproj_b(qc + 1, wp, "xq", psC, "cps")
                    prev = (cpair, qc)
                # drain the last q chunk's output projection
                emit_outproj(prev[0], prev[1], range(D // 128), tail=True)

    nc.finalize()
    return nc


_NC_CACHE = None


def _get_nc():
    global _NC_CACHE
    if _NC_CACHE is None:
        _NC_CACHE = build_nc()
    return _NC_CACHE


def shard_inputs(query, key, value, mask, wq, bq, wk, bk, wv, bv, wo, bo):
    """Build the per-core input maps (host-side shard prep)."""
    import ml_dtypes

    query = np.asarray(query, np.float32)
    key = np.asarray(key, np.float32)
    value = np.asarray(value, np.float32)
    mask = np.asarray(mask)
    wq = np.asarray(wq, np.float32); bq = np.asarray(bq, np.float32)
    wk = np.asarray(wk, np.float32); bk = np.asarray(bk, np.float32)
    wv = np.asarray(wv, np.float32); bv = np.asarray(bv, np.float32)
    wo = np.asarray(wo, np.float32); bo = np.asarray(bo, np.float32)

    in_maps = []
    maskT_b = [np.ascontiguousarray(mask[b].T).astype(ml_dtypes.bfloat16)
               for b in range(B)]
    xT = {}
    for b in range(B):
        xT[b] = (
            np.ascontiguousarray(query[b].T),
            np.ascontiguousarray(key[b].T),
            np.ascontiguousarray(value[b].T),
        )
    for c in range(NCORES):
        b, hg = divmod(c, HG)
        sl = slice(hg * DH, (hg + 1) * DH)
        wo_block = wo[:, sl]                       # [1024, 512]
        v_r = bv[sl] @ wo_block.T                  # [1024]
        if hg == 0:
            v_r = v_r + bo
        in_maps.append({
            "xqT": xT[b][0],
            "xkT": xT[b][1],
            "xvT": xT[b][2],
            "maskT": maskT_b[b],
            "wqT": np.ascontiguousarray(wq[sl].T),
            "wkT": np.ascontiguousarray(wk[sl].T),
            "wvT": np.ascontiguousarray(wv[sl].T),
            "woT": np.ascontiguousarray(wo_block.T),
            "bq2": np.ascontiguousarray((bq[sl] / 8.0).reshape(DH // 128, 128).T),
            "bk2": np.ascontiguousarray(bk[sl].reshape(DH // 128, 128).T),
            "vr2": np.ascontiguousarray(v_r.reshape(D // 128, 128).T),
        })
    return in_maps


def combine_outputs(results):
    """results: list of per-core {"outT": [1024, 2048]} -> full [B, S, D]."""
    out = np.empty((B, S, D), np.float32)
    for b in range(B):
        acc = results[2 * b]["outT"] + results[2 * b + 1]["outT"]
        out[b] = acc.T
    return out


def kernel(**inputs):
    from concourse.bass_utils import run_bass_kernel_spmd

    nc = _get_nc()
    in_maps = shard_inputs(**inputs)
    res = run_bass_kernel_spmd(nc, in_maps, list(range(NCORES)))
    return combine_outputs(res.results)


# revision 31
# speedup vs baseline: 1.0115x; 1.0115x over previous
"""Multi-head attention Trainium2 Bass kernel, sharded over 8 NeuronCores.

Problem: B=4, S=2048, D=1024, H=16 heads (DK=64), fp32, random 0/1 mask.

Sharding (data-parallel batch x tensor-parallel heads):
  core c handles batch b = c // 2, head-group hg = c % 2 (8 heads = 512 dims).
  Each core computes Q/K/V projections for its head-group, masked softmax
  attention for its 8 heads, and a partial output projection over its 512
  contraction dims. The host sums the two partials per batch (the "all-reduce"
  is a host-side pairwise add since we gather outputs anyway).

On-device layout (per core):
  All matmuls run as float32r (full PE rate at N=512, ~1e-3 relative precision)
  except attention-weights @ V which runs in bf16 (E and V tiles), because the
  mask multiply on the vector engine needs bf16 for its 2x mode.

  K^T is kept resident [512, 2048] (head dim on partitions); Q^T is computed
  per 512-wide q-chunk inside the attention loop (hides the Q projection under
  the exp-bound phase). Scores come out transposed, S^T = [k, q], with the two
  heads of a pair row-packed into disjoint halves of the PE array. The softmax
  sum over k rides the P@V matmul as a ones column appended to V (row 64 of
  the C' accumulator = sum_k E_masked). exp() runs on the scalar engine out of
  PSUM (2 banks per ACTIVATE); the mask multiply runs on the vector engine in
  bf16 2x mode; 1/Z = scalar-engine partition-shifting copy (PSUM row 64 ->
  SBUF row 0) + reciprocal_approx_fast + gpsimd partition_broadcast. The
  output projection of chunk qc is emitted inside chunk qc+1's loop so its
  matmuls never stall the in-order PE queue.

  Output is produced transposed ([1024, 2048] per core); host re-transposes
  and sums the two head-group partials per batch.
"""
import numpy as np

import concourse.bass as bass
import concourse.mybir as mybir
import concourse.tile as tile
from concourse import bacc

B, S, D, H = 4, 2048, 1024, 16
DK = D // H          # 64
NCORES = 8
HG = 2               # head groups (tensor-parallel factor per batch)
HPG = H // HG        # 8 heads per core
DH = D // HG         # 512 head dims per core
QCN = 4              # q chunks
QCS = S // QCN       # 512
KT = S // 128        # 16 k tiles
DT = D // 128        # 8 contraction tiles for projections
F32 = mybir.dt.float32
F32R = mybir.dt.float32r
BF16 = mybir.dt.bfloat16

# k-tile grouping for the exp pass (PSUM banks per S^T buffer)
EXP_GROUPS = [3, 3, 3, 3, 2, 2]
assert sum(EXP_GROUPS) == KT


def r(ap):
    """Matmul operands are stored as float32r already; identity."""
    return ap


def build_nc():
    nc = bacc.Bacc(None)
    xqT = nc.declare_dram_parameter("xqT", [D, S], F32R, isOutput=False)
    xkT = nc.declare_dram_parameter("xkT", [D, S], F32R, isOutput=False)
    xvT = nc.declare_dram_parameter("xvT", [D, S], F32R, isOutput=False)
    maskT = nc.declare_dram_parameter("maskT", [S, S], BF16, isOutput=False)
    wqT = nc.declare_dram_parameter("wqT", [D, DH], F32R, isOutput=False)
    wkT = nc.declare_dram_parameter("wkT", [D, DH], F32R, isOutput=False)
    wvT = nc.declare_dram_parameter("wvT", [D, DH], F32R, isOutput=False)
    woT = nc.declare_dram_parameter("woT", [DH, D], F32R, isOutput=False)
    bq2 = nc.declare_dram_parameter("bq2", [128, DH // 128], F32, isOutput=False)
    bk2 = nc.declare_dram_parameter("bk2", [128, DH // 128], F32, isOutput=False)
    vr2 = nc.declare_dram_parameter("vr2", [128, D // 128], F32, isOutput=False)
    outT = nc.declare_dram_parameter("outT", [D, S], F32, isOutput=True)

    with tile.TileContext(nc) as tc:
        with tc.tile_pool(name="persist", bufs=1) as pp:
            # ---- persistent tiles ----
            wo_full = pp.tile([128, DH // 128, D], F32R, tag="wo", name="wo_full")
            wo_sb = [wo_full[:, i, :] for i in range(DH // 128)]
            bias_sb = pp.tile([128, 2 * (DH // 128) + D // 128], F32, tag="bias",
                              name="bias_sb")
            bq_sb = bias_sb[:, 0:DH // 128]
            bk_sb = bias_sb[:, DH // 128:2 * (DH // 128)]
            vr_sb = bias_sb[:, 2 * (DH // 128):]
            wq_full = pp.tile([128, DT, DH], F32R, tag="wq", name="wq_full")
            wq_sb = [wq_full[:, i, :] for i in range(DT)]
            kt_sb = [pp.tile([128, S], F32R, tag=f"kt{i}", name=f"kt{i}")
                     for i in range(DH // 128)]
            v_full = pp.tile([128, KT, HPG * 65], BF16, tag="v", name="v_full")
            v_sb = [v_full[:, i, :] for i in range(KT)]

            # ---- phase A: projections ----
            with (
                tc.tile_pool(name="w_a", bufs=1) as wpool,
                tc.tile_pool(name="x_a", bufs=2) as xpool,
                tc.tile_pool(name="ps_a", bufs=6, space="PSUM") as pspool,
            ):
                wk_full = wpool.tile([128, DT, DH], F32R, tag="wk", name="wk_full")
                wv_full = wpool.tile([128, DT, DH], F32R, tag="wv", name="wv_full")
                wk_sb = [wk_full[:, i, :] for i in range(DT)]
                wv_sb = [wv_full[:, i, :] for i in range(DT)]
                # wk first on the sync queue (K projection runs first)
                for i in range(DT):
                    nc.sync.dma_start(wk_sb[i][:], wkT[i * 128:(i + 1) * 128, :])
                nc.gpsimd.dma_start(bq_sb[:, :], bq2[:])
                nc.gpsimd.dma_start(bk_sb[:, :], bk2[:])
                nc.gpsimd.dma_start(vr_sb[:, :], vr2[:])
                for i in range(KT):
                    ones_cols = v_sb[i].rearrange("p (h c) -> p h c", h=HPG)[:, :, 64:65]
                    nc.gpsimd.memset(ones_cols, 1.0)

                def qk_proj(xT_dram, w_sb, dst_tiles, scale, bias_sb, wtag,
                            first_on_pool=False):
                    for qc in range(QCN):
                        x_t = xpool.tile([128, DT, QCS], F32R, tag="x", name=f"x_{wtag}",
                                         bufs=2)
                        xs = xT_dram[:, qc * QCS:(qc + 1) * QCS].rearrange(
                            "(t p) s -> p t s", p=128)
                        for i in range(DT):
                            eng = nc.gpsimd if (first_on_pool or i % 2) else nc.sync
                            eng.dma_start(x_t[:, i:i + 1, :], xs[:, i:i + 1, :])
                        first_on_pool = False
                        ps_l = [pspool.tile([128, QCS], F32, tag="ps",
                                            name=f"ps_{wtag}{dt}")
                                for dt in range(DH // 128)]
                        for i in range(DT):
                            for dt in range(DH // 128):
                                nc.tensor.matmul(
                                    ps_l[dt][:], w_sb[i][:, dt * 128:(dt + 1) * 128],
                                    x_t[:, i, :], start=(i == 0), stop=(i == DT - 1))
                        for dt in range(DH // 128):
                            nc.vector.tensor_scalar(
                                dst_tiles[dt][:, qc * QCS:(qc + 1) * QCS], ps_l[dt][:],
                                scale, bias_sb[:, dt:dt + 1],
                                mybir.AluOpType.mult, mybir.AluOpType.add)

                qk_proj(xkT, wk_sb, kt_sb, 1.0, bk_sb, "k", first_on_pool=True)
                for i in range(DT):
                    nc.sync.dma_start(wv_sb[i][:], wvT[i * 128:(i + 1) * 128, :])
                for i in range(DT):
                    nc.sync.dma_start(wq_sb[i][:], wqT[i * 128:(i + 1) * 128, :])

                def qproj_b(qc, xp, xtag, psp, pstag):
                    qt_t = pp.tile([128, DH // 128, QCS], F32R, tag="qt",
                                   name="qt_t", bufs=2)
                    xq2 = [xp.tile([128, DT // 2, QCS], F32R, tag=xtag, name="xq2",
                                   bufs=2) for _ in range(2)]
                    for c in range(2):
                        xs = xqT[c * (D // 2):(c + 1) * (D // 2),
                                 qc * QCS:(qc + 1) * QCS].rearrange(
                            "(t p) s -> p t s", p=128)
                        nc.sync.dma_start(xq2[c][:], xs)
                    for dt in range(DH // 128):
                        ps = psp.tile([128, QCS], F32, tag=pstag, name="ps_q")
                        for i in range(DT):
                            nc.tensor.matmul(
                                ps[:], wq_sb[i][:, dt * 128:(dt + 1) * 128],
                                xq2[i // (DT // 2)][:, i % (DT // 2), :],
                                start=(i == 0), stop=(i == DT - 1))
                        nc.vector.tensor_scalar(
                            qt_t[:, dt, :], ps[:], 0.125, bq_sb[:, dt:dt + 1],
                            mybir.AluOpType.mult, mybir.AluOpType.add)
                    qt_tiles[qc] = qt_t

                qt_tiles = {}

                for st4 in range(KT // 4):
                    xv4 = xpool.tile([128, DT, QCS], F32R, tag="x", name="xv4", bufs=2)
                    nc.gpsimd.dma_start(
                        xv4[:],
                        xvT[:, st4 * QCS:(st4 + 1) * QCS].rearrange(
                            "(t p) s -> p t s", p=128))
                    for sub in range(4):
                        st = st4 * 4 + sub
                        ps = pspool.tile([128, DH], F32, tag="ps", name="ps_v")
                        for i in range(DT):
                            nc.tensor.matmul(
                                ps[:], xv4[:, i, sub * 128:(sub + 1) * 128],
                                wv_sb[i][:], start=(i == 0), stop=(i == DT - 1))
                        vdst = v_sb[st].rearrange("p (h c) -> p h c", h=HPG)[:, :, 0:64]
                        nc.vector.tensor_copy(
                            vdst, ps[:].rearrange("p (h c) -> p h c", h=HPG))
                qproj_b(0, xpool, "x", pspool, "ps")

            # wo loads can land any time before the first output projection
            for i in range(DH // 128):
                nc.gpsimd.dma_start(wo_sb[i][:], woT[i * 128:(i + 1) * 128, :])

            # ---- phase B: attention + pipelined output projection ----
            with (
                tc.tile_pool(name="work", bufs=2) as wp,
                tc.tile_pool(name="psS", bufs=2, space="PSUM") as psS,
                tc.tile_pool(name="psC", bufs=4, space="PSUM") as psC,
            ):
                prev = None  # (cpair tiles, qc) pending output projection

                def emit_outproj(cpair, qc, ots, tail=False):
                    for ot in ots:
                        po = psC.tile([128, QCS], F32, tag="cps", name="po")
                        for j in range(HPG // 2):
                            nc.tensor.matmul(
                                po[:], wo_sb[j][:, ot * 128:(ot + 1) * 128],
                                cpair[j][:],
                                start=(j == 0), stop=(j == HPG // 2 - 1))
                        o_sb = wp.tile([128, QCS], F32, tag="o", name="o_sb", bufs=3)
                        if tail and ot % 2 == 0:
                            nc.scalar.activation(
                                o_sb[:], po[:], mybir.ActivationFunctionType.Identity,
                                bias=vr_sb[:, ot:ot + 1])
                        else:
                            nc.vector.tensor_scalar(
                                o_sb[:], po[:], 1.0, vr_sb[:, ot:ot + 1],
                                mybir.AluOpType.mult, mybir.AluOpType.add)
                        eng = nc.gpsimd if ot % 2 else nc.sync
                        eng.dma_start(
                            outT[ot * 128:(ot + 1) * 128, qc * QCS:(qc + 1) * QCS],
                            o_sb[:])

                def load_mask(qc):
                    mask_sb = wp.tile([128, KT, QCS], BF16, tag="mask", name="mask_sb",
                                      bufs=2)
                    ms = maskT[:, qc * QCS:(qc + 1) * QCS].rearrange(
                        "(t p) s -> p t s", p=128)
                    hm = KT // 2
                    nc.sync.dma_start(mask_sb[:, 0:hm, :], ms[:, 0:hm, :])
                    nc.gpsimd.dma_start(mask_sb[:, hm:KT, :], ms[:, hm:KT, :])
                    return mask_sb

                mask_next = load_mask(0)
                for qc in range(QCN):
                    mask_sb = mask_next
                    qt_cur = qt_tiles.pop(qc)
                    cpair_t = wp.tile([128, HPG // 2, QCS], F32R, tag="cp",
                                      name="cpair_t", bufs=2)
                    cpair = [cpair_t[:, j, :] for j in range(HPG // 2)]
                    for j in range(HPG // 2):
                        dtile = j
                        cps = [psC.tile([128, QCS], F32, tag="cps", name=f"cps{hh}",
                                        bufs=4) for hh in range(2)]
                        for kt in range(KT):
                            sps = psS.tile([128, 2, QCS], F32, tag="sps", name="sps",
                                           bufs=2)
                            for hh in range(2):
                                prow = hh * 64
                                nc.tensor.matmul(
                                    sps[:, hh, :],
                                    kt_sb[dtile][prow:prow + 64, kt * 128:(kt + 1) * 128],
                                    qt_cur[prow:prow + 64, dtile, :],
                                    start=True, stop=True)
                            e_sb = wp.tile([128, 2, QCS], BF16, tag="e", name="e_sb",
                                           bufs=4)
                            nc.scalar.activation(
                                e_sb[:], sps[:],
                                mybir.ActivationFunctionType.Exp)
                            meng = nc.gpsimd if kt % 4 == 3 else nc.vector
                            for hh in range(2):
                                meng.tensor_mul(
                                    e_sb[:, hh, :], e_sb[:, hh, :], mask_sb[:, kt, :])
                            for hh in range(2):
                                h = 2 * j + hh
                                nc.tensor.matmul(
                                    cps[hh][0:65, :],
                                    v_sb[kt][:, h * 65:(h + 1) * 65],
                                    e_sb[:, hh, :],
                                    start=(kt == 0), stop=(kt == KT - 1))
                        for hh in (1, 0):
                            # normalize: C[d, q] / Z[q]; Z = PSUM row 64.
                            # The scalar engine does a partition-SHIFTING copy
                            # (PSUM row 64 -> SBUF row 0), reciprocal runs at
                            # partition 0 (custom-DVE PSUM reads at partition
                            # 64 are broken on HW), then broadcast + multiply.
                            ns = wp.tile([64, 2, QCS], F32, tag="ns", name="ns", bufs=2)
                            nc.scalar.copy(ns[0:1, 0, :], cps[hh][64:65, :])
                            nc.vector.reciprocal_approx_fast(
                                out=ns[0:1, 1, :], in_=ns[0:1, 0, :])
                            rb = ns[0:64, 0, :]
                            nc.gpsimd.partition_broadcast(rb, ns[0:1, 1, :],
                                                          channels=64)
                            if hh == 0:
                                nc.vector.tensor_mul(cpair[j][0:64, :],
                                                     cps[hh][0:64, :], rb)
                            else:
                                cstage = wp.tile([64, QCS], F32R, tag="cstage",
                                                 name="cstage", bufs=3)
                                nc.vector.tensor_mul(cstage[:], cps[hh][0:64, :], rb)
                                nc.sync.dma_start(cpair[j][64:128, :], cstage[:])
                        if prev is not None:
                            emit_outproj(prev[0], prev[1], range(2 * j, 2 * j + 2))
                        if j == 0 and qc + 1 < QCN:
                            mask_next = load_mask(qc + 1)
                        if j == 1 and qc + 1 < QCN:
                            qproj_b(qc + 1, wp, "xq", psC, "cps")
                    prev = (cpair, qc)
                # drain the last q chunk's output projection
                emit_outproj(prev[0], prev[1], range(D // 128), tail=True)

    nc.finalize()
    return nc


_NC_CACHE = None


def _get_nc():
    global _NC_CACHE
    if _NC_CACHE is None:
        _NC_CACHE = build_nc()
    return _NC_CACHE


def shard_inputs(query, key, value, mask, wq, bq, wk, bk, wv, bv, wo, bo):
    """Build the per-core input maps (host-side shard prep)."""
    import ml_dtypes

    query = np.asarray(query, np.float32)
    key = np.asarray(key, np.float32)
    value = np.asarray(value, np.float32)
    mask = np.asarray(mask)
    wq = np.asarray(wq, np.float32); bq = np.asarray(bq, np.float32)
    wk = np.asarray(wk, np.float32); bk = np.asarray(bk, np.float32)
    wv = np.asarray(wv, np.float32); bv = np.asarray(bv, np.float32)
    wo = np.asarray(wo, np.float32); bo = np.asarray(bo, np.float32)

    in_maps = []
    maskT_b = [np.ascontiguousarray(mask[b].T).astype(ml_dtypes.bfloat16)
               for b in range(B)]
    xT = {}
    for b in range(B):
        xT[b] = (
            np.ascontiguousarray(query[b].T),
            np.ascontiguousarray(key[b].T),
            np.ascontiguousarray(value[b].T),
        )
    for c in range(NCORES):
        b, hg = divmod(c, HG)
        sl = slice(hg * DH, (hg + 1) * DH)
        wo_block = wo[:, sl]                       # [1024, 512]
        v_r = bv[sl] @ wo_block.T                  # [1024]
        if hg == 0:
            v_r = v_r + bo
        in_maps.append({
            "xqT": xT[b][0],
            "xkT": xT[b][1],
            "xvT": xT[b][2],
            "maskT": maskT_b[b],
            "wqT": np.ascontiguousarray(wq[sl].T),
            "wkT": np.ascontiguousarray(wk[sl].T),
            "wvT": np.ascontiguousarray(wv[sl].T),
            "woT": np.ascontiguousarray(wo_block.T),
            "bq2": np.ascontiguousarray((bq[sl] / 8.0).reshape(DH // 128, 128).T),
            "bk2": np.ascontiguousarray(bk[sl].reshape(DH // 128, 128).T),
            "vr2": np.ascontiguousarray(v_r.reshape(D // 128, 128).T),
        })
    return in_maps


def combine_outputs(results):
    """results: list of per-core {"outT": [1024, 2048]} -> full [B, S, D]."""
    out = np.empty((B, S, D), np.float32)
    for b in range(B):
        acc = results[2 * b]["outT"] + results[2 * b + 1]["outT"]
        out[b] = acc.T
    return out


def kernel(**inputs):
    from concourse.bass_utils import run_bass_kernel_spmd

    nc = _get_nc()
    in_maps = shard_inputs(**inputs)
    res = run_bass_kernel_spmd(nc, in_maps, list(range(NCORES)))
    return combine_outputs(res.results)
